# revision 16
# baseline (speedup 1.0000x reference)
"""BoundaryConvLayer GNN message-passing kernel for 8 Trainium2 NeuronCores.

Math (reference):
    alpha = relu(x @ dir_w.T + dir_b); beta = relu(x @ neu_w.T + neu_b)
    gamma = x @ rob_w.T + rob_b;       h    = x @ fc_w.T + fc_b
    agg   = segment_sum(h[row] + h[col], row)
    out   = (beta * agg + gamma) / (alpha + beta * degree + EPS)

Restructure (v2): by linearity of h,
    agg = deg*h2 + S @ fc_w.T   with  S = segment_sum(x[col], row),
          h2 = x @ fc_w.T + 2*fc_b
so the gather moves raw x rows (fp8, 64B/row) instead of fp16 h rows and no
h-table ever has to be built in DRAM.  fc_w is applied after the segment-sum
via a PE transpose (S -> S.T) + matmul (S.T as lhsT against fc_w.T).

Distribution: nodes sharded 8 ways by contiguous row range; edges partitioned
by row owner so the segment-sum is core-local; the fp8 x-table is replicated
(host-written, no device build).  Local rows are sorted by degree (desc) so
each 128-row block has near-uniform edge count; messages accumulate 4 lanes
per block with identity-stationary matmuls into f32 PSUM.  alpha/beta/gamma/h2
are computed in f32: the relu sign decision feeds a 1/(...+1e-8) denominator,
so fp16 pre-activations would blow up near relu zero-crossings.  The fp8
gather only degrades S (the neighbour sum), whose error is negligible against
the output scale set by the near-singular denominators.
"""

import functools
import os
import sys

import numpy as np

if "/opt/trn_rl_repo" not in sys.path:
    sys.path.insert(0, "/opt/trn_rl_repo")

EPS = 1e-8
P = 128


def _cfg_full():
    return dict(
        N=100_000,
        D=64,
        NCORES=8,
        GROUP=4,  # blocks per gather/psum group
    )


def _derive(cfg):
    N, NCORES = cfg["N"], cfg["NCORES"]
    NLOC = N // NCORES
    NBLK = -(-NLOC // P)
    NLOC_PAD = NBLK * P
    NTAB = N + 128  # fp8 x-table rows; row ZROW=N is zeros for pad gathers
    cfg.update(NLOC=NLOC, NBLK=NBLK, NLOC_PAD=NLOC_PAD, NTAB=NTAB, ZROW=N)
    return cfg


def _host_prep(cfg, x, edge_index, degree):
    """Build per-core input maps + unshard metadata."""
    import ml_dtypes

    N, D, NCORES = cfg["N"], cfg["D"], cfg["NCORES"]
    NLOC, NBLK = cfg["NLOC"], cfg["NBLK"]
    NLOC_PAD, NTAB, ZROW = cfg["NLOC_PAD"], cfg["NTAB"], cfg["ZROW"]

    x = np.asarray(x, np.float32)
    row = np.asarray(edge_index[0], np.int64)
    col = np.asarray(edge_index[1], np.int64)
    deg_in = np.asarray(degree, np.float32).reshape(-1)

    # replicated fp8 gather table of raw node features
    x8 = np.zeros((NTAB, D), ml_dtypes.float8_e4m3)
    x8[:N] = x.astype(ml_dtypes.float8_e4m3)

    cores = []
    dmax_all = np.zeros((NCORES, NBLK), np.int64)
    for k in range(NCORES):
        base = k * NLOC
        m = (row >= base) & (row < base + NLOC)
        r = row[m] - base
        c = col[m]
        counts = np.bincount(r, minlength=NLOC)
        perm = np.argsort(-counts, kind="stable")
        rank = np.empty(NLOC, np.int64)
        rank[perm] = np.arange(NLOC)
        rr = rank[r]
        order = np.argsort(rr, kind="stable")
        rs = rr[order]
        cs = c[order]
        dsort = counts[perm]
        starts = np.zeros(NLOC, np.int64)
        np.cumsum(dsort[:-1], out=starts[1:])
        occ = np.arange(len(rs)) - starts[rs]
        dmax = np.zeros(NBLK, np.int64)
        for b in range(NBLK):
            seg = dsort[b * P:(b + 1) * P]
            dmax[b] = seg.max() if len(seg) else 0
        dmax_all[k] = dmax
        cores.append(dict(base=base, perm=perm, rs=rs, cs=cs, occ=occ))

    # >=2 so both PSUM accumulation lanes get a start write
    colw = np.maximum(dmax_all.max(axis=0), 2).astype(np.int64)
    coff = np.zeros(NBLK, np.int64)
    np.cumsum(colw[:-1], out=coff[1:])
    K_total = int(colw.sum())
    cfg["colw"] = [int(v) for v in colw]
    cfg["K_total"] = K_total

    in_maps = []
    for k in range(NCORES):
        cc = cores[k]
        base, perm = cc["base"], cc["perm"]
        eidx = np.full((P, K_total), ZROW, np.int32)
        b = cc["rs"] // P
        pp = cc["rs"] % P
        kcol = coff[b] + cc["occ"]
        eidx[pp, kcol] = cc["cs"]

        xt_loc = np.zeros((D + 1, NLOC_PAD), np.float32)
        xt_loc[:D, :NLOC] = x[base:base + NLOC][perm].T
        xt_loc[D, :NLOC] = 1.0

        dpad = np.zeros(NLOC_PAD, np.float32)
        dpad[:NLOC] = deg_in[base:base + NLOC][perm]
        degm = np.ascontiguousarray(dpad.reshape(NBLK, P).T)  # [p, b]

        in_maps.append({
            "x8": x8,
            "xt_loc": xt_loc,
            "eidx": eidx,
            "degm": degm,
        })
    return in_maps, cores


def _host_weights(cfg, fc_w, fc_b, dir_w, dir_b, neu_w, neu_b, rob_w, rob_b):
    import ml_dtypes

    D = cfg["D"]
    wcat = np.zeros((D + 1, 4 * D), np.float32)
    for t, (w, bb) in enumerate([(dir_w, dir_b), (neu_w, neu_b),
                                 (rob_w, rob_b), (fc_w, fc_b)]):
        wcat[:D, t * D:(t + 1) * D] = np.asarray(w, np.float32).T
        wcat[D, t * D:(t + 1) * D] = np.asarray(bb, np.float32)
    # h2 = x @ fc_w.T + 2*fc_b absorbs the deg*fc_b term from the col sum
    wcat[D, 3 * D:4 * D] *= 2.0
    wfcT = np.ascontiguousarray(np.asarray(fc_w, np.float32).T).astype(
        np.float16)  # [d, d'] so  S @ fc_w.T = (S.T as lhsT) x wfcT
    ident8 = np.eye(P, dtype=ml_dtypes.float8_e4m3)
    return wcat, wfcT, ident8


def _build_nc(cfg):
    import concourse.bass as bass
    import concourse.bacc as bacc
    import concourse.mybir as mybir
    import concourse.tile as tile
    from concourse.masks import make_identity

    D = cfg["D"]
    NBLK, NLOC_PAD = cfg["NBLK"], cfg["NLOC_PAD"]
    NTAB = cfg["NTAB"]
    K_total, colw, GROUP = cfg["K_total"], cfg["colw"], cfg["GROUP"]
    f32, f16, i32 = mybir.dt.float32, mybir.dt.float16, mybir.dt.int32
    bf16 = mybir.dt.bfloat16
    f8 = mybir.dt.float8e4
    coff = np.zeros(NBLK, np.int64)
    np.cumsum(np.asarray(colw[:-1]), out=coff[1:])

    nc = bacc.Bacc()
    x8_d = nc.declare_dram_parameter("x8", [NTAB, D], f8, isOutput=False)
    xt_loc_d = nc.declare_dram_parameter("xt_loc", [D + 1, NLOC_PAD], f32,
                                         isOutput=False)
    eidx_d = nc.declare_dram_parameter("eidx", [P, K_total], i32,
                                       isOutput=False)
    degm_d = nc.declare_dram_parameter("degm", [P, NBLK], f32, isOutput=False)
    wcat_d = nc.declare_dram_parameter("wcat", [D + 1, 4 * D], f32,
                                       isOutput=False)
    wfcT_d = nc.declare_dram_parameter("wfcT", [D, D], f16, isOutput=False)
    ident8_d = nc.declare_dram_parameter("ident8", [P, P], f8, isOutput=False)
    y_d = nc.declare_dram_parameter("y", [P, NBLK * D], bf16,
                                    isOutput=True)

    # edge groups: <=GROUP blocks and bounded gather width per group
    groups, cur, csum = [], [], 0
    for b in range(NBLK):
        if cur and (csum + colw[b] > 17 * GROUP or len(cur) >= GROUP):
            groups.append(cur)
            cur, csum = [], 0
        cur.append(b)
        csum += colw[b]
    if cur:
        groups.append(cur)
    NG = len(groups)

    with tile.TileContext(nc) as tc:
        with (
            tc.tile_pool(name="const", bufs=1) as cp,
            tc.tile_pool(name="xtl", bufs=3) as xtlp,
            tc.tile_pool(name="msg", bufs=4) as mp,
            tc.tile_pool(name="ssb", bufs=2) as sp,
            tc.tile_pool(name="stb", bufs=2) as stp,
            tc.tile_pool(name="tmp", bufs=2) as tp,
            tc.tile_pool(name="osb", bufs=2) as op,
            tc.tile_pool(name="ps1", bufs=2, space="PSUM") as pp1,
            tc.tile_pool(name="ps2", bufs=2, space="PSUM") as pp2,
            tc.tile_pool(name="psT", bufs=2, space="PSUM") as ppT,
            tc.tile_pool(name="psA", bufs=2, space="PSUM") as ppA,
        ):
            # loop-invariant constants: loaded once, resident across LOOPR
            eidx_sb = cp.tile([P, K_total], i32)
            nc.sync.dma_start(out=eidx_sb[:], in_=eidx_d[:])
            wcat = cp.tile([D + 1, 4 * D], f32)
            nc.sync.dma_start(out=wcat[:], in_=wcat_d[:])
            wfcT = cp.tile([D, D], f16)
            nc.sync.dma_start(out=wfcT[:], in_=wfcT_d[:])
            ident8 = cp.tile([P, P], f8)
            nc.sync.dma_start(out=ident8[:], in_=ident8_d[:])
            degm_sb = cp.tile([P, NBLK], f32)
            nc.sync.dma_start(out=degm_sb[:], in_=degm_d[:])
            ident16 = cp.tile([P, P], f16)
            make_identity(nc, ident16[:])
            abgh = cp.tile([P, NBLK * 4 * D], f32)
            abgh3 = abgh[:].rearrange("p (t c) -> p t c", c=4 * D)

            def _bodyfn():

                # all gathers up-front on the Pool queue; msg pool bufs
                # pipeline them against PE consumption
                msgs = []
                for gi, blocks in enumerate(groups):
                    b0 = blocks[0]
                    goff = int(coff[b0])
                    Kg = int(sum(colw[b] for b in blocks))
                    msg = mp.tile([P, 17 * GROUP * D], f8, tag="msg")
                    nc.gpsimd.indirect_dma_start(
                        out=msg[:, :Kg * D], out_offset=None,
                        in_=x8_d[:],
                        in_offset=bass.IndirectOffsetOnAxis(
                            ap=eidx_sb[:, goff:goff + Kg], axis=0),
                    )
                    msgs.append(msg)

                # phase-1b work list: alpha/beta/gamma/h2 for pairs of blocks,
                # interleaved between gather groups to fill PE gaps
                XCH = 8
                xt_tiles = {}

                def emit_1b_pair(pi):
                    t0 = 2 * pi
                    c0 = (t0 // XCH) * XCH
                    if c0 not in xt_tiles:
                        nb_c = min(XCH, NBLK - c0)
                        xt = xtlp.tile([D + 1, XCH * P], f32, tag="xtl")
                        nc.sync.dma_start(
                            out=xt[:, :nb_c * P],
                            in_=xt_loc_d[:, P * c0:P * (c0 + nb_c)])
                        xt_tiles[c0] = xt
                    xt = xt_tiles[c0]
                    ps = pp1.tile([P, 2 * 4 * D], f32, tag="ps1b")
                    for j in range(2):
                        t = t0 + j
                        if t >= NBLK:
                            continue
                        nc.tensor.matmul(
                            out=ps[:, j * 4 * D:(j + 1) * 4 * D],
                            lhsT=xt[:, P * (t - c0):P * (t - c0 + 1)],
                            rhs=wcat[:], start=True, stop=True,
                            skip_group_check=True)
                    nlive = min(2, NBLK - t0)
                    ps3 = ps[:].rearrange("p (t c) -> p t c", c=4 * D)
                    out3 = abgh3[:, t0:t0 + nlive]
                    # relu on alpha|beta (EPS folded into den later); both
                    # halves on the Activation engine to keep DVE free
                    nc.scalar.activation(
                        out=out3[:, :, 0:2 * D], in_=ps3[:, :nlive, 0:2 * D],
                        func=mybir.ActivationFunctionType.Relu)
                    nc.scalar.copy(out=out3[:, :, 2 * D:4 * D],
                                   in_=ps3[:, :nlive, 2 * D:4 * D])

                NPAIR = (NBLK + 1) // 2
                pairs_per_g = -(-NPAIR // NG)
                next_pair = 0

                # steady-state pipeline over groups
                state = {}

                def emit_segsum(gi):
                    blocks = groups[gi]
                    nb = len(blocks)
                    msg = msgs[gi]
                    ps = pp2.tile([P, GROUP * 2 * D], f32, tag="psagg")
                    kk = 0
                    for bi, b in enumerate(blocks):
                        w = colw[b]
                        nj = (w + 1) // 2
                        for j in range(nj):
                            ncols = min(2, w - 2 * j)
                            nc.tensor.matmul(
                                out=ps[:, (2 * bi) * D:(2 * bi + ncols) * D],
                                lhsT=ident8[:],
                                rhs=msg[:, (kk + 2 * j) * D:
                                        (kk + 2 * j + ncols) * D],
                                start=(j == 0), stop=(j == nj - 1),
                                skip_group_check=True)
                        kk += w
                    # combine lanes -> S in f16 (copy + add; one PSUM input
                    # per DVE instruction)
                    ps4 = ps[:].rearrange("p (t l c) -> p t l c", l=2, c=D)
                    s_sb = sp.tile([P, GROUP * D], f16, tag="ssb")
                    s3 = s_sb[:].rearrange("p (t c) -> p t c", c=D)
                    nc.vector.tensor_copy(out=s3[:, :nb], in_=ps4[:, :nb, 0])
                    nc.vector.tensor_tensor(out=s3[:, :nb], in0=s3[:, :nb],
                                            in1=ps4[:, :nb, 1],
                                            op=mybir.AluOpType.add)
                    state[gi] = dict(s_sb=s_sb, nb=nb, blocks=blocks)

                def emit_finish(gi):
                    st = state.pop(gi)
                    s_sb, nb, blocks = st["s_sb"], st["nb"], st["blocks"]
                    b0 = blocks[0]
                    # transpose S per block:  psT[:, bi] = S_bi.T
                    psT = ppT.tile([2 * D, GROUP * P], f32, tag="psT")
                    for bi in range(nb):
                        nc.tensor.matmul(
                            out=psT[:D, bi * P:(bi + 1) * P],
                            lhsT=s_sb[:, bi * D:(bi + 1) * D],
                            rhs=ident16[:], start=True, stop=True,
                            skip_group_check=True)
                    st_sb = stp.tile([D, GROUP * P], f16, tag="stb")
                    nc.scalar.copy(out=st_sb[:, :nb * P], in_=psT[:D, :nb * P])
                    # apply fc_w:  A_bi = S_bi @ fc_w.T
                    psA = ppA.tile([P, GROUP * D], f32, tag="psA")
                    for bi in range(nb):
                        nc.tensor.matmul(
                            out=psA[:, bi * D:(bi + 1) * D],
                            lhsT=st_sb[:, bi * P:(bi + 1) * P],
                            rhs=wfcT[:], start=True, stop=True,
                            skip_group_check=True)
                    # epilogue
                    a3 = psA[:].rearrange("p (t c) -> p t c", c=D)
                    num = tp.tile([P, GROUP * D], f32, tag="num")
                    den = tp.tile([P, GROUP * D], f32, tag="den")
                    num3 = num[:].rearrange("p (t c) -> p t c", c=D)
                    den3 = den[:].rearrange("p (t c) -> p t c", c=D)
                    bsl = abgh3[:, b0:b0 + nb, D:2 * D]
                    gsl = abgh3[:, b0:b0 + nb, 2 * D:3 * D]
                    asl = abgh3[:, b0:b0 + nb, 0:D]
                    degb = degm_sb[:, b0:b0 + nb].rearrange(
                        "p (t u) -> p t u", u=1).to_broadcast([P, nb, D])
                    nn3 = num3[:, :nb]
                    dd3 = den3[:, :nb]
                    # den = alpha + beta*deg + EPS (group-wide ops)
                    nc.vector.tensor_tensor(out=dd3, in0=bsl, in1=degb,
                                            op=mybir.AluOpType.mult)
                    nc.vector.tensor_tensor(out=dd3, in0=dd3, in1=asl,
                                            op=mybir.AluOpType.add)
                    nc.vector.tensor_scalar(
                        out=den[:, :nb * D], in0=den[:, :nb * D],
                        scalar1=EPS, scalar2=None, op0=mybir.AluOpType.add)
                    nc.vector.reciprocal(out=dd3, in_=dd3)
                    # num = beta * (deg*h2 + S@fc_w.T) + gamma
                    # deg*h2 on the Activation engine (per-partition scale)
                    for bi, b in enumerate(blocks):
                        nc.scalar.mul(out=num[:, bi * D:(bi + 1) * D],
                                      in_=abgh3[:, b, 3 * D:4 * D],
                                      mul=degm_sb[:, b:b + 1])
                    nc.vector.tensor_tensor(out=nn3, in0=nn3, in1=a3[:, :nb],
                                            op=mybir.AluOpType.add)
                    nc.vector.tensor_tensor(out=nn3, in0=nn3, in1=bsl,
                                            op=mybir.AluOpType.mult)
                    nc.vector.tensor_tensor(out=nn3, in0=nn3, in1=gsl,
                                            op=mybir.AluOpType.add)
                    osb = op.tile([P, GROUP * D], bf16, tag="osb")
                    osb3 = osb[:].rearrange("p (t c) -> p t c", c=D)
                    nc.vector.tensor_tensor(out=osb3[:, :nb], in0=nn3,
                                            in1=dd3, op=mybir.AluOpType.mult)
                    nc.sync.dma_start(
                        out=y_d[:, b0 * D:(b0 + nb) * D], in_=osb[:, :nb * D])

                prev = None
                for gi in range(NG):
                    for _ in range(pairs_per_g):
                        if next_pair < NPAIR:
                            emit_1b_pair(next_pair)
                            next_pair += 1
                    emit_segsum(gi)
                    if prev is not None:
                        emit_finish(prev)
                    prev = gi
                while next_pair < NPAIR:
                    emit_1b_pair(next_pair)
                    next_pair += 1
                emit_finish(prev)

            LOOPR = cfg.get("LOOPR", 0)
            if LOOPR:
                with tc.For_i(0, LOOPR, 1) as _i:
                    _bodyfn()
            else:
                _bodyfn()
    nc.finalize()
    return nc


_BUILD_CACHE = {}
LAST_PROFILE = {}


def _get_runner(cfg):
    """Compile the bass program once; return an executor over 8 cores.

    Mirrors concourse.bass2jax.run_bass_via_pjrt's multi-core branch but
    caches the jitted callable so repeated executions don't re-trace."""
    key = (cfg["N"], cfg["NCORES"], tuple(cfg["colw"]), cfg["GROUP"],
           cfg.get("LOOPR", 0))
    if key in _BUILD_CACHE:
        return _BUILD_CACHE[key]

    import jax
    import concourse.mybir as mybir
    from jax.experimental.shard_map import shard_map
    from jax.sharding import Mesh, PartitionSpec
    from concourse.bass2jax import (
        _bass_exec_p, install_neuronx_cc_hook, partition_id_tensor)

    nc = _build_nc(cfg)
    install_neuronx_cc_hook()
    n_cores = cfg["NCORES"]
    partition_name = (nc.partition_id_tensor.name
                      if nc.partition_id_tensor else None)
    in_names, out_names, out_avals, zero_outs = [], [], [], []
    for alloc in nc.m.functions[0].allocations:
        if not isinstance(alloc, mybir.MemoryLocationSet):
            continue
        name = alloc.memorylocations[0].name
        if alloc.kind == "ExternalInput":
            if name != partition_name:
                in_names.append(name)
        elif alloc.kind == "ExternalOutput":
            out_names.append(name)
            shape = tuple(alloc.tensor_shape)
            dtype = mybir.dt.np(alloc.dtype)
            out_avals.append(jax.core.ShapedArray(shape, dtype))
            zero_outs.append(np.zeros(shape, dtype))
    n_params = len(in_names)
    n_outs = len(out_avals)
    all_names = in_names + out_names
    if partition_name is not None:
        all_names.append(partition_name)

    def _body(*args):
        operands = list(args)
        if partition_name is not None:
            operands.append(partition_id_tensor())
        return tuple(_bass_exec_p.bind(
            *operands,
            out_avals=tuple(out_avals),
            in_names=tuple(all_names),
            out_names=tuple(out_names),
            lowering_input_output_aliases=(),
            sim_require_finite=True,
            sim_require_nnan=True,
            nc=nc,
        ))

    devices = jax.devices()[:n_cores]
    mesh = Mesh(np.asarray(devices), ("core",))
    in_specs = (PartitionSpec("core"),) * (n_params + n_outs)
    out_specs = (PartitionSpec("core"),) * n_outs
    donate = tuple(range(n_params, n_params + n_outs))
    sharded = jax.jit(
        shard_map(_body, mesh=mesh, in_specs=in_specs, out_specs=out_specs,
                  check_rep=False),
        donate_argnums=donate, keep_unused=True)

    import jax.numpy as jnp

    from jax.sharding import NamedSharding
    _zshard = tuple(NamedSharding(mesh, PartitionSpec("core"))
                    for _ in zero_outs)

    @functools.partial(jax.jit, out_shardings=_zshard)
    def _mkzeros():
        return tuple(jnp.zeros((n_cores * z.shape[0], *z.shape[1:]), z.dtype)
                     for z in zero_outs)

    def run(in_maps, reps=1, async_reps=0):
        import time as _time
        per_core = [[np.asarray(m[n]) for n in in_names] for m in in_maps]
        concat_in = [np.concatenate([per_core[c][i] for c in range(n_cores)],
                                    axis=0) for i in range(n_params)]
        concat_in = [jax.device_put(a) for a in concat_in]
        for a in concat_in:
            a.block_until_ready()
        times = []
        out_arrs = None
        for _ in range(max(1, reps)):
            concat_zeros = _mkzeros()
            for z in concat_zeros:
                z.block_until_ready()
            t0 = _time.perf_counter()
            out_arrs = sharded(*concat_in, *concat_zeros)
            for o in out_arrs:
                o.block_until_ready()
            times.append(_time.perf_counter() - t0)
        if async_reps:
            zsets = []
            for _ in range(async_reps):
                zs = _mkzeros()
                for z in zs:
                    z.block_until_ready()
                zsets.append(zs)
            t0 = _time.perf_counter()
            pend = [sharded(*concat_in, *zs) for zs in zsets]
            for oa in pend:
                for o in oa:
                    o.block_until_ready()
            times.append(("async_avg",
                          (_time.perf_counter() - t0) / async_reps))
        results = [
            {name: np.asarray(out_arrs[i]).reshape(n_cores,
                                                   *out_avals[i].shape)[c]
             for i, name in enumerate(out_names)}
            for c in range(n_cores)
        ]
        return results, times

    _BUILD_CACHE[key] = run
    return run


def _prepare(cfg, x, edge_index, degree, fc_w, fc_b, dir_w, dir_b,
             neu_w, neu_b, rob_w, rob_b):
    x = np.asarray(x)
    in_maps, cores = _host_prep(cfg, x, edge_index, degree)
    wcat, wfcT, ident8 = _host_weights(cfg, fc_w, fc_b, dir_w, dir_b,
                                       neu_w, neu_b, rob_w, rob_b)
    for im in in_maps:
        im["wcat"] = wcat
        im["wfcT"] = wfcT
        im["ident8"] = ident8
    return in_maps, cores


def _unshard(cfg, results, cores):
    N, D, NLOC, NBLK = cfg["N"], cfg["D"], cfg["NLOC"], cfg["NBLK"]
    out = np.empty((N, D), np.float32)
    for k in range(cfg["NCORES"]):
        y2 = np.asarray(results[k]["y"], np.float32).reshape(P, NBLK, D)
        y = np.ascontiguousarray(y2.transpose(1, 0, 2)).reshape(-1, D)[:NLOC]
        cc = cores[k]
        out[cc["base"] + cc["perm"]] = y
    return out


def kernel(x, edge_index, degree, fc_w, fc_b, dir_w, dir_b,
           neu_w, neu_b, rob_w, rob_b, _cfg=None, _reps=1, _async=0):
    cfg = _derive(dict(_cfg) if _cfg is not None else _cfg_full())
    in_maps, cores = _prepare(cfg, x, edge_index, degree, fc_w, fc_b,
                              dir_w, dir_b, neu_w, neu_b, rob_w, rob_b)
    run = _get_runner(cfg)
    results, times = run(in_maps, reps=_reps, async_reps=_async)
    LAST_PROFILE.clear()
    LAST_PROFILE["wall_times_s"] = times
    sync_times = [t for t in times if not isinstance(t, tuple)]
    LAST_PROFILE["exec_time_ns"] = int(min(sync_times) * 1e9)
    return _unshard(cfg, results, cores)


# revision 17
# speedup vs baseline: 537.0961x; 537.0961x over previous
"""BoundaryConvLayer GNN message-passing kernel for 8 Trainium2 NeuronCores.

Math (reference):
    alpha = relu(x @ dir_w.T + dir_b); beta = relu(x @ neu_w.T + neu_b)
    gamma = x @ rob_w.T + rob_b;       h    = x @ fc_w.T + fc_b
    agg   = segment_sum(h[row] + h[col], row)
    out   = (beta * agg + gamma) / (alpha + beta * degree + EPS)

Restructure (v2): by linearity of h,
    agg = deg*h2 + S @ fc_w.T   with  S = segment_sum(x[col], row),
          h2 = x @ fc_w.T + 2*fc_b
so the gather moves raw x rows (fp8, 64B/row) instead of fp16 h rows and no
h-table ever has to be built in DRAM.  fc_w is applied after the segment-sum
via a PE transpose (S -> S.T) + matmul (S.T as lhsT against fc_w.T).

Distribution: nodes sharded 8 ways by contiguous row range; edges partitioned
by row owner so the segment-sum is core-local; the fp8 x-table is replicated
(host-written, no device build).  Local rows are sorted by degree (desc) so
each 128-row block has near-uniform edge count; messages accumulate 4 lanes
per block with identity-stationary matmuls into f32 PSUM.  alpha/beta/gamma/h2
are computed in f32: the relu sign decision feeds a 1/(...+1e-8) denominator,
so fp16 pre-activations would blow up near relu zero-crossings.  The fp8
gather only degrades S (the neighbour sum), whose error is negligible against
the output scale set by the near-singular denominators.
"""

import functools
import os
import sys

import numpy as np

if "/opt/trn_rl_repo" not in sys.path:
    sys.path.insert(0, "/opt/trn_rl_repo")

EPS = 1e-8
P = 128


def _cfg_full():
    return dict(
        N=100_000,
        D=64,
        NCORES=8,
        GROUP=4,  # blocks per gather/psum group
    )


def _derive(cfg):
    N, NCORES = cfg["N"], cfg["NCORES"]
    NLOC = N // NCORES
    NBLK = -(-NLOC // P)
    NLOC_PAD = NBLK * P
    NTAB = N + 128  # fp8 x-table rows; row ZROW=N is zeros for pad gathers
    cfg.update(NLOC=NLOC, NBLK=NBLK, NLOC_PAD=NLOC_PAD, NTAB=NTAB, ZROW=N)
    return cfg


def _host_prep(cfg, x, edge_index, degree):
    """Build per-core input maps + unshard metadata."""
    import ml_dtypes

    N, D, NCORES = cfg["N"], cfg["D"], cfg["NCORES"]
    NLOC, NBLK = cfg["NLOC"], cfg["NBLK"]
    NLOC_PAD, NTAB, ZROW = cfg["NLOC_PAD"], cfg["NTAB"], cfg["ZROW"]

    x = np.asarray(x, np.float32)
    row = np.asarray(edge_index[0], np.int64)
    col = np.asarray(edge_index[1], np.int64)
    deg_in = np.asarray(degree, np.float32).reshape(-1)

    # replicated fp8 gather table of raw node features
    x8 = np.zeros((NTAB, D), ml_dtypes.float8_e4m3)
    x8[:N] = x.astype(ml_dtypes.float8_e4m3)

    cores = []
    dmax_all = np.zeros((NCORES, NBLK), np.int64)
    for k in range(NCORES):
        base = k * NLOC
        m = (row >= base) & (row < base + NLOC)
        r = row[m] - base
        c = col[m]
        counts = np.bincount(r, minlength=NLOC)
        perm = np.argsort(-counts, kind="stable")
        rank = np.empty(NLOC, np.int64)
        rank[perm] = np.arange(NLOC)
        rr = rank[r]
        order = np.argsort(rr, kind="stable")
        rs = rr[order]
        cs = c[order]
        dsort = counts[perm]
        starts = np.zeros(NLOC, np.int64)
        np.cumsum(dsort[:-1], out=starts[1:])
        occ = np.arange(len(rs)) - starts[rs]
        dmax = np.zeros(NBLK, np.int64)
        for b in range(NBLK):
            seg = dsort[b * P:(b + 1) * P]
            dmax[b] = seg.max() if len(seg) else 0
        dmax_all[k] = dmax
        cores.append(dict(base=base, perm=perm, rs=rs, cs=cs, occ=occ))

    # >=2 so both PSUM accumulation lanes get a start write
    colw = np.maximum(dmax_all.max(axis=0), 2).astype(np.int64)
    coff = np.zeros(NBLK, np.int64)
    np.cumsum(colw[:-1], out=coff[1:])
    K_total = int(colw.sum())
    cfg["colw"] = [int(v) for v in colw]
    cfg["K_total"] = K_total

    in_maps = []
    for k in range(NCORES):
        cc = cores[k]
        base, perm = cc["base"], cc["perm"]
        eidx = np.full((P, K_total), ZROW, np.int32)
        b = cc["rs"] // P
        pp = cc["rs"] % P
        kcol = coff[b] + cc["occ"]
        eidx[pp, kcol] = cc["cs"]

        xt_loc = np.zeros((D + 1, NLOC_PAD), np.float32)
        xt_loc[:D, :NLOC] = x[base:base + NLOC][perm].T
        xt_loc[D, :NLOC] = 1.0

        dpad = np.zeros(NLOC_PAD, np.float32)
        dpad[:NLOC] = deg_in[base:base + NLOC][perm]
        degm = np.ascontiguousarray(dpad.reshape(NBLK, P).T)  # [p, b]

        in_maps.append({
            "x8": x8,
            "xt_loc": xt_loc,
            "eidx": eidx,
            "degm": degm,
        })
    return in_maps, cores


def _host_weights(cfg, fc_w, fc_b, dir_w, dir_b, neu_w, neu_b, rob_w, rob_b):
    import ml_dtypes

    D = cfg["D"]
    wcat = np.zeros((D + 1, 4 * D), np.float32)
    for t, (w, bb) in enumerate([(dir_w, dir_b), (neu_w, neu_b),
                                 (rob_w, rob_b), (fc_w, fc_b)]):
        wcat[:D, t * D:(t + 1) * D] = np.asarray(w, np.float32).T
        wcat[D, t * D:(t + 1) * D] = np.asarray(bb, np.float32)
    # h2 = x @ fc_w.T + 2*fc_b absorbs the deg*fc_b term from the col sum
    wcat[D, 3 * D:4 * D] *= 2.0
    wfcT = np.ascontiguousarray(np.asarray(fc_w, np.float32).T).astype(
        np.float16)  # [d, d'] so  S @ fc_w.T = (S.T as lhsT) x wfcT
    ident8 = np.eye(P, dtype=ml_dtypes.float8_e4m3)
    return wcat, wfcT, ident8


def _build_nc(cfg):
    import concourse.bass as bass
    import concourse.bacc as bacc
    import concourse.mybir as mybir
    import concourse.tile as tile
    from concourse.masks import make_identity

    D = cfg["D"]
    NBLK, NLOC_PAD = cfg["NBLK"], cfg["NLOC_PAD"]
    NTAB = cfg["NTAB"]
    K_total, colw, GROUP = cfg["K_total"], cfg["colw"], cfg["GROUP"]
    f32, f16, i32 = mybir.dt.float32, mybir.dt.float16, mybir.dt.int32
    bf16 = mybir.dt.bfloat16
    f8 = mybir.dt.float8e4
    coff = np.zeros(NBLK, np.int64)
    np.cumsum(np.asarray(colw[:-1]), out=coff[1:])

    nc = bacc.Bacc()
    x8_d = nc.declare_dram_parameter("x8", [NTAB, D], f8, isOutput=False)
    xt_loc_d = nc.declare_dram_parameter("xt_loc", [D + 1, NLOC_PAD], f32,
                                         isOutput=False)
    eidx_d = nc.declare_dram_parameter("eidx", [P, K_total], i32,
                                       isOutput=False)
    degm_d = nc.declare_dram_parameter("degm", [P, NBLK], f32, isOutput=False)
    wcat_d = nc.declare_dram_parameter("wcat", [D + 1, 4 * D], f32,
                                       isOutput=False)
    wfcT_d = nc.declare_dram_parameter("wfcT", [D, D], f16, isOutput=False)
    ident8_d = nc.declare_dram_parameter("ident8", [P, P], f8, isOutput=False)
    y_d = nc.declare_dram_parameter("y", [P, NBLK * D], bf16,
                                    isOutput=True)

    # edge groups: <=GROUP blocks and bounded gather width per group
    groups, cur, csum = [], [], 0
    for b in range(NBLK):
        if cur and (csum + colw[b] > 17 * GROUP or len(cur) >= GROUP):
            groups.append(cur)
            cur, csum = [], 0
        cur.append(b)
        csum += colw[b]
    if cur:
        groups.append(cur)
    NG = len(groups)

    with tile.TileContext(nc) as tc:
        with (
            tc.tile_pool(name="const", bufs=1) as cp,
            tc.tile_pool(name="xtl", bufs=3) as xtlp,
            tc.tile_pool(name="msg", bufs=4) as mp,
            tc.tile_pool(name="ssb", bufs=2) as sp,
            tc.tile_pool(name="stb", bufs=2) as stp,
            tc.tile_pool(name="tmp", bufs=2) as tp,
            tc.tile_pool(name="osb", bufs=2) as op,
            tc.tile_pool(name="ps1", bufs=2, space="PSUM") as pp1,
            tc.tile_pool(name="ps2", bufs=2, space="PSUM") as pp2,
            tc.tile_pool(name="psT", bufs=2, space="PSUM") as ppT,
            tc.tile_pool(name="psA", bufs=2, space="PSUM") as ppA,
        ):
            # loop-invariant constants: loaded once, resident across LOOPR
            eidx_sb = cp.tile([P, K_total], i32)
            nc.sync.dma_start(out=eidx_sb[:], in_=eidx_d[:])
            wcat = cp.tile([D + 1, 4 * D], f32)
            nc.sync.dma_start(out=wcat[:], in_=wcat_d[:])
            wfcT = cp.tile([D, D], f16)
            nc.sync.dma_start(out=wfcT[:], in_=wfcT_d[:])
            ident8 = cp.tile([P, P], f8)
            nc.sync.dma_start(out=ident8[:], in_=ident8_d[:])
            degm_sb = cp.tile([P, NBLK], f32)
            nc.sync.dma_start(out=degm_sb[:], in_=degm_d[:])
            ident16 = cp.tile([P, P], f16)
            make_identity(nc, ident16[:])
            abgh = cp.tile([P, NBLK * 4 * D], f32)
            abgh3 = abgh[:].rearrange("p (t c) -> p t c", c=4 * D)

            def _bodyfn():

                # all gathers up-front on the Pool queue; msg pool bufs
                # pipeline them against PE consumption
                msgs = []
                for gi, blocks in enumerate(groups):
                    b0 = blocks[0]
                    goff = int(coff[b0])
                    Kg = int(sum(colw[b] for b in blocks))
                    msg = mp.tile([P, 17 * GROUP * D], f8, tag="msg")
                    nc.gpsimd.indirect_dma_start(
                        out=msg[:, :Kg * D], out_offset=None,
                        in_=x8_d[:],
                        in_offset=bass.IndirectOffsetOnAxis(
                            ap=eidx_sb[:, goff:goff + Kg], axis=0),
                    )
                    msgs.append(msg)

                # phase-1b work list: alpha/beta/gamma/h2 for pairs of blocks,
                # interleaved between gather groups to fill PE gaps
                XCH = 8
                xt_tiles = {}

                def emit_1b_pair(pi):
                    t0 = 2 * pi
                    c0 = (t0 // XCH) * XCH
                    if c0 not in xt_tiles:
                        nb_c = min(XCH, NBLK - c0)
                        xt = xtlp.tile([D + 1, XCH * P], f32, tag="xtl")
                        nc.sync.dma_start(
                            out=xt[:, :nb_c * P],
                            in_=xt_loc_d[:, P * c0:P * (c0 + nb_c)])
                        xt_tiles[c0] = xt
                    xt = xt_tiles[c0]
                    ps = pp1.tile([P, 2 * 4 * D], f32, tag="ps1b")
                    for j in range(2):
                        t = t0 + j
                        if t >= NBLK:
                            continue
                        nc.tensor.matmul(
                            out=ps[:, j * 4 * D:(j + 1) * 4 * D],
                            lhsT=xt[:, P * (t - c0):P * (t - c0 + 1)],
                            rhs=wcat[:], start=True, stop=True,
                            skip_group_check=True)
                    nlive = min(2, NBLK - t0)
                    ps3 = ps[:].rearrange("p (t c) -> p t c", c=4 * D)
                    out3 = abgh3[:, t0:t0 + nlive]
                    # relu on alpha|beta (EPS folded into den later); both
                    # halves on the Activation engine to keep DVE free
                    nc.scalar.activation(
                        out=out3[:, :, 0:2 * D], in_=ps3[:, :nlive, 0:2 * D],
                        func=mybir.ActivationFunctionType.Relu)
                    nc.scalar.copy(out=out3[:, :, 2 * D:4 * D],
                                   in_=ps3[:, :nlive, 2 * D:4 * D])

                NPAIR = (NBLK + 1) // 2
                pairs_per_g = -(-NPAIR // NG)
                next_pair = 0

                # steady-state pipeline over groups
                state = {}

                def emit_segsum(gi):
                    blocks = groups[gi]
                    nb = len(blocks)
                    msg = msgs[gi]
                    ps = pp2.tile([P, GROUP * 2 * D], f32, tag="psagg")
                    kk = 0
                    for bi, b in enumerate(blocks):
                        w = colw[b]
                        nj = (w + 1) // 2
                        for j in range(nj):
                            ncols = min(2, w - 2 * j)
                            nc.tensor.matmul(
                                out=ps[:, (2 * bi) * D:(2 * bi + ncols) * D],
                                lhsT=ident8[:],
                                rhs=msg[:, (kk + 2 * j) * D:
                                        (kk + 2 * j + ncols) * D],
                                start=(j == 0), stop=(j == nj - 1),
                                skip_group_check=True)
                        kk += w
                    # combine lanes -> S in f16 (copy + add; one PSUM input
                    # per DVE instruction)
                    ps4 = ps[:].rearrange("p (t l c) -> p t l c", l=2, c=D)
                    s_sb = sp.tile([P, GROUP * D], f16, tag="ssb")
                    s3 = s_sb[:].rearrange("p (t c) -> p t c", c=D)
                    nc.vector.tensor_copy(out=s3[:, :nb], in_=ps4[:, :nb, 0])
                    nc.vector.tensor_tensor(out=s3[:, :nb], in0=s3[:, :nb],
                                            in1=ps4[:, :nb, 1],
                                            op=mybir.AluOpType.add)
                    state[gi] = dict(s_sb=s_sb, nb=nb, blocks=blocks)

                def emit_finish(gi):
                    st = state.pop(gi)
                    s_sb, nb, blocks = st["s_sb"], st["nb"], st["blocks"]
                    b0 = blocks[0]
                    # transpose S per block:  psT[:, bi] = S_bi.T
                    psT = ppT.tile([2 * D, GROUP * P], f32, tag="psT")
                    for bi in range(nb):
                        nc.tensor.matmul(
                            out=psT[:D, bi * P:(bi + 1) * P],
                            lhsT=s_sb[:, bi * D:(bi + 1) * D],
                            rhs=ident16[:], start=True, stop=True,
                            skip_group_check=True)
                    st_sb = stp.tile([D, GROUP * P], f16, tag="stb")
                    nc.scalar.copy(out=st_sb[:, :nb * P], in_=psT[:D, :nb * P])
                    # apply fc_w:  A_bi = S_bi @ fc_w.T
                    psA = ppA.tile([P, GROUP * D], f32, tag="psA")
                    for bi in range(nb):
                        nc.tensor.matmul(
                            out=psA[:, bi * D:(bi + 1) * D],
                            lhsT=st_sb[:, bi * P:(bi + 1) * P],
                            rhs=wfcT[:], start=True, stop=True,
                            skip_group_check=True)
                    # epilogue
                    a3 = psA[:].rearrange("p (t c) -> p t c", c=D)
                    num = tp.tile([P, GROUP * D], f32, tag="num")
                    den = tp.tile([P, GROUP * D], f32, tag="den")
                    num3 = num[:].rearrange("p (t c) -> p t c", c=D)
                    den3 = den[:].rearrange("p (t c) -> p t c", c=D)
                    bsl = abgh3[:, b0:b0 + nb, D:2 * D]
                    gsl = abgh3[:, b0:b0 + nb, 2 * D:3 * D]
                    asl = abgh3[:, b0:b0 + nb, 0:D]
                    degb = degm_sb[:, b0:b0 + nb].rearrange(
                        "p (t u) -> p t u", u=1).to_broadcast([P, nb, D])
                    nn3 = num3[:, :nb]
                    dd3 = den3[:, :nb]
                    # den = alpha + beta*deg + EPS (group-wide ops)
                    nc.vector.tensor_tensor(out=dd3, in0=bsl, in1=degb,
                                            op=mybir.AluOpType.mult)
                    nc.vector.tensor_tensor(out=dd3, in0=dd3, in1=asl,
                                            op=mybir.AluOpType.add)
                    nc.vector.tensor_scalar(
                        out=den[:, :nb * D], in0=den[:, :nb * D],
                        scalar1=EPS, scalar2=None, op0=mybir.AluOpType.add)
                    nc.vector.reciprocal(out=dd3, in_=dd3)
                    # num = beta * (deg*h2 + S@fc_w.T) + gamma
                    # deg*h2 on the Activation engine (per-partition scale)
                    for bi, b in enumerate(blocks):
                        nc.scalar.mul(out=num[:, bi * D:(bi + 1) * D],
                                      in_=abgh3[:, b, 3 * D:4 * D],
                                      mul=degm_sb[:, b:b + 1])
                    nc.vector.tensor_tensor(out=nn3, in0=nn3, in1=a3[:, :nb],
                                            op=mybir.AluOpType.add)
                    nc.vector.tensor_tensor(out=nn3, in0=nn3, in1=bsl,
                                            op=mybir.AluOpType.mult)
                    nc.vector.tensor_tensor(out=nn3, in0=nn3, in1=gsl,
                                            op=mybir.AluOpType.add)
                    osb = op.tile([P, GROUP * D], bf16, tag="osb")
                    osb3 = osb[:].rearrange("p (t c) -> p t c", c=D)
                    nc.vector.tensor_tensor(out=osb3[:, :nb], in0=nn3,
                                            in1=dd3, op=mybir.AluOpType.mult)
                    nc.sync.dma_start(
                        out=y_d[:, b0 * D:(b0 + nb) * D], in_=osb[:, :nb * D])

                prev = None
                for gi in range(NG):
                    for _ in range(pairs_per_g):
                        if next_pair < NPAIR:
                            emit_1b_pair(next_pair)
                            next_pair += 1
                    # finish the previous group before the next segsum so its
                    # apply-matmul (psA) lands ahead of the new PE burst
                    if prev is not None:
                        emit_finish(prev)
                    emit_segsum(gi)
                    prev = gi
                while next_pair < NPAIR:
                    emit_1b_pair(next_pair)
                    next_pair += 1
                emit_finish(prev)

            LOOPR = cfg.get("LOOPR", 0)
            if LOOPR:
                with tc.For_i(0, LOOPR, 1) as _i:
                    _bodyfn()
            else:
                _bodyfn()
    nc.finalize()
    return nc


_BUILD_CACHE = {}
LAST_PROFILE = {}


def _get_runner(cfg):
    """Compile the bass program once; return an executor over 8 cores.

    Mirrors concourse.bass2jax.run_bass_via_pjrt's multi-core branch but
    caches the jitted callable so repeated executions don't re-trace."""
    key = (cfg["N"], cfg["NCORES"], tuple(cfg["colw"]), cfg["GROUP"],
           cfg.get("LOOPR", 0))
    if key in _BUILD_CACHE:
        return _BUILD_CACHE[key]

    import jax
    import concourse.mybir as mybir
    from jax.experimental.shard_map import shard_map
    from jax.sharding import Mesh, PartitionSpec
    from concourse.bass2jax import (
        _bass_exec_p, install_neuronx_cc_hook, partition_id_tensor)

    nc = _build_nc(cfg)
    install_neuronx_cc_hook()
    n_cores = cfg["NCORES"]
    partition_name = (nc.partition_id_tensor.name
                      if nc.partition_id_tensor else None)
    in_names, out_names, out_avals, zero_outs = [], [], [], []
    for alloc in nc.m.functions[0].allocations:
        if not isinstance(alloc, mybir.MemoryLocationSet):
            continue
        name = alloc.memorylocations[0].name
        if alloc.kind == "ExternalInput":
            if name != partition_name:
                in_names.append(name)
        elif alloc.kind == "ExternalOutput":
            out_names.append(name)
            shape = tuple(alloc.tensor_shape)
            dtype = mybir.dt.np(alloc.dtype)
            out_avals.append(jax.core.ShapedArray(shape, dtype))
            zero_outs.append(np.zeros(shape, dtype))
    n_params = len(in_names)
    n_outs = len(out_avals)
    all_names = in_names + out_names
    if partition_name is not None:
        all_names.append(partition_name)

    def _body(*args):
        operands = list(args)
        if partition_name is not None:
            operands.append(partition_id_tensor())
        return tuple(_bass_exec_p.bind(
            *operands,
            out_avals=tuple(out_avals),
            in_names=tuple(all_names),
            out_names=tuple(out_names),
            lowering_input_output_aliases=(),
            sim_require_finite=True,
            sim_require_nnan=True,
            nc=nc,
        ))

    devices = jax.devices()[:n_cores]
    mesh = Mesh(np.asarray(devices), ("core",))
    in_specs = (PartitionSpec("core"),) * (n_params + n_outs)
    out_specs = (PartitionSpec("core"),) * n_outs
    donate = tuple(range(n_params, n_params + n_outs))
    sharded = jax.jit(
        shard_map(_body, mesh=mesh, in_specs=in_specs, out_specs=out_specs,
                  check_rep=False),
        donate_argnums=donate, keep_unused=True)

    import jax.numpy as jnp

    from jax.sharding import NamedSharding
    _zshard = tuple(NamedSharding(mesh, PartitionSpec("core"))
                    for _ in zero_outs)

    @functools.partial(jax.jit, out_shardings=_zshard)
    def _mkzeros():
        return tuple(jnp.zeros((n_cores * z.shape[0], *z.shape[1:]), z.dtype)
                     for z in zero_outs)

    def run(in_maps, reps=1, async_reps=0):
        import time as _time
        per_core = [[np.asarray(m[n]) for n in in_names] for m in in_maps]
        concat_in = [np.concatenate([per_core[c][i] for c in range(n_cores)],
                                    axis=0) for i in range(n_params)]
        concat_in = [jax.device_put(a) for a in concat_in]
        for a in concat_in:
            a.block_until_ready()
        times = []
        out_arrs = None
        for _ in range(max(1, reps)):
            concat_zeros = _mkzeros()
            for z in concat_zeros:
                z.block_until_ready()
            t0 = _time.perf_counter()
            out_arrs = sharded(*concat_in, *concat_zeros)
            for o in out_arrs:
                o.block_until_ready()
            times.append(_time.perf_counter() - t0)
        if async_reps:
            zsets = []
            for _ in range(async_reps):
                zs = _mkzeros()
                for z in zs:
                    z.block_until_ready()
                zsets.append(zs)
            t0 = _time.perf_counter()
            pend = [sharded(*concat_in, *zs) for zs in zsets]
            for oa in pend:
                for o in oa:
                    o.block_until_ready()
            times.append(("async_avg",
                          (_time.perf_counter() - t0) / async_reps))
        results = [
            {name: np.asarray(out_arrs[i]).reshape(n_cores,
                                                   *out_avals[i].shape)[c]
             for i, name in enumerate(out_names)}
            for c in range(n_cores)
        ]
        return results, times

    _BUILD_CACHE[key] = run
    return run


def _prepare(cfg, x, edge_index, degree, fc_w, fc_b, dir_w, dir_b,
             neu_w, neu_b, rob_w, rob_b):
    x = np.asarray(x)
    in_maps, cores = _host_prep(cfg, x, edge_index, degree)
    wcat, wfcT, ident8 = _host_weights(cfg, fc_w, fc_b, dir_w, dir_b,
                                       neu_w, neu_b, rob_w, rob_b)
    for im in in_maps:
        im["wcat"] = wcat
        im["wfcT"] = wfcT
        im["ident8"] = ident8
    return in_maps, cores


def _unshard(cfg, results, cores):
    N, D, NLOC, NBLK = cfg["N"], cfg["D"], cfg["NLOC"], cfg["NBLK"]
    out = np.empty((N, D), np.float32)
    for k in range(cfg["NCORES"]):
        y2 = np.asarray(results[k]["y"], np.float32).reshape(P, NBLK, D)
        y = np.ascontiguousarray(y2.transpose(1, 0, 2)).reshape(-1, D)[:NLOC]
        cc = cores[k]
        out[cc["base"] + cc["perm"]] = y
    return out


def kernel(x, edge_index, degree, fc_w, fc_b, dir_w, dir_b,
           neu_w, neu_b, rob_w, rob_b, _cfg=None, _reps=1, _async=0):
    cfg = _derive(dict(_cfg) if _cfg is not None else _cfg_full())
    in_maps, cores = _prepare(cfg, x, edge_index, degree, fc_w, fc_b,
                              dir_w, dir_b, neu_w, neu_b, rob_w, rob_b)
    run = _get_runner(cfg)
    results, times = run(in_maps, reps=_reps, async_reps=_async)
    LAST_PROFILE.clear()
    LAST_PROFILE["wall_times_s"] = times
    sync_times = [t for t in times if not isinstance(t, tuple)]
    LAST_PROFILE["exec_time_ns"] = int(min(sync_times) * 1e9)
    return _unshard(cfg, results, cores)


# revision 18
# speedup vs baseline: 935.6745x; 1.7421x over previous
"""BoundaryConvLayer GNN message-passing kernel for 8 Trainium2 NeuronCores.

Math (reference):
    alpha = relu(x @ dir_w.T + dir_b); beta = relu(x @ neu_w.T + neu_b)
    gamma = x @ rob_w.T + rob_b;       h    = x @ fc_w.T + fc_b
    agg   = segment_sum(h[row] + h[col], row)
    out   = (beta * agg + gamma) / (alpha + beta * degree + EPS)

Restructure (v2): by linearity of h,
    agg = deg*h2 + S @ fc_w.T   with  S = segment_sum(x[col], row),
          h2 = x @ fc_w.T + 2*fc_b
so the gather moves raw x rows (fp8, 64B/row) instead of fp16 h rows and no
h-table ever has to be built in DRAM.  fc_w is applied after the segment-sum
via a PE transpose (S -> S.T) + matmul (S.T as lhsT against fc_w.T).

Distribution: nodes sharded 8 ways by contiguous row range; edges partitioned
by row owner so the segment-sum is core-local; the fp8 x-table is replicated
(host-written, no device build).  Local rows are sorted by degree (desc) so
each 128-row block has near-uniform edge count; messages accumulate 4 lanes
per block with identity-stationary matmuls into f32 PSUM.  alpha/beta/gamma/h2
are computed in f32: the relu sign decision feeds a 1/(...+1e-8) denominator,
so fp16 pre-activations would blow up near relu zero-crossings.  The fp8
gather only degrades S (the neighbour sum), whose error is negligible against
the output scale set by the near-singular denominators.
"""

import functools
import os
import sys

import numpy as np

if "/opt/trn_rl_repo" not in sys.path:
    sys.path.insert(0, "/opt/trn_rl_repo")

EPS = 1e-8
P = 128


def _cfg_full():
    return dict(
        N=100_000,
        D=64,
        NCORES=8,
        GROUP=4,  # blocks per gather/psum group
        DMASCRATCH=49152,  # SWDGE descriptor-ring carveout (bytes)
    )


def _derive(cfg):
    N, NCORES = cfg["N"], cfg["NCORES"]
    NLOC = N // NCORES
    NBLK = -(-NLOC // P)
    NLOC_PAD = NBLK * P
    NTAB = N + 128  # fp8 x-table rows; row ZROW=N is zeros for pad gathers
    cfg.update(NLOC=NLOC, NBLK=NBLK, NLOC_PAD=NLOC_PAD, NTAB=NTAB, ZROW=N)
    return cfg


def _host_prep(cfg, x, edge_index, degree):
    """Build per-core input maps + unshard metadata."""
    import ml_dtypes

    N, D, NCORES = cfg["N"], cfg["D"], cfg["NCORES"]
    NLOC, NBLK = cfg["NLOC"], cfg["NBLK"]
    NLOC_PAD, NTAB, ZROW = cfg["NLOC_PAD"], cfg["NTAB"], cfg["ZROW"]

    x = np.asarray(x, np.float32)
    row = np.asarray(edge_index[0], np.int64)
    col = np.asarray(edge_index[1], np.int64)
    deg_in = np.asarray(degree, np.float32).reshape(-1)

    # replicated fp8 gather table of raw node features
    x8 = np.zeros((NTAB, D), ml_dtypes.float8_e4m3)
    x8[:N] = x.astype(ml_dtypes.float8_e4m3)

    cores = []
    dmax_all = np.zeros((NCORES, NBLK), np.int64)
    for k in range(NCORES):
        base = k * NLOC
        m = (row >= base) & (row < base + NLOC)
        r = row[m] - base
        c = col[m]
        counts = np.bincount(r, minlength=NLOC)
        perm = np.argsort(-counts, kind="stable")
        rank = np.empty(NLOC, np.int64)
        rank[perm] = np.arange(NLOC)
        rr = rank[r]
        order = np.argsort(rr, kind="stable")
        rs = rr[order]
        cs = c[order]
        dsort = counts[perm]
        starts = np.zeros(NLOC, np.int64)
        np.cumsum(dsort[:-1], out=starts[1:])
        occ = np.arange(len(rs)) - starts[rs]
        dmax = np.zeros(NBLK, np.int64)
        for b in range(NBLK):
            seg = dsort[b * P:(b + 1) * P]
            dmax[b] = seg.max() if len(seg) else 0
        dmax_all[k] = dmax
        cores.append(dict(base=base, perm=perm, rs=rs, cs=cs, occ=occ))

    # >=2 so both PSUM accumulation lanes get a start write
    colw = np.maximum(dmax_all.max(axis=0), 2).astype(np.int64)
    coff = np.zeros(NBLK, np.int64)
    np.cumsum(colw[:-1], out=coff[1:])
    K_total = int(colw.sum())
    cfg["colw"] = [int(v) for v in colw]
    cfg["K_total"] = K_total

    in_maps = []
    for k in range(NCORES):
        cc = cores[k]
        base, perm = cc["base"], cc["perm"]
        eidx = np.full((P, K_total), ZROW, np.int32)
        b = cc["rs"] // P
        pp = cc["rs"] % P
        kcol = coff[b] + cc["occ"]
        eidx[pp, kcol] = cc["cs"]

        xt_loc = np.zeros((D + 1, NLOC_PAD), np.float32)
        xt_loc[:D, :NLOC] = x[base:base + NLOC][perm].T
        xt_loc[D, :NLOC] = 1.0

        dpad = np.zeros(NLOC_PAD, np.float32)
        dpad[:NLOC] = deg_in[base:base + NLOC][perm]
        degm = np.ascontiguousarray(dpad.reshape(NBLK, P).T)  # [p, b]

        in_maps.append({
            "x8": x8,
            "xt_loc": xt_loc,
            "eidx": eidx,
            "degm": degm,
        })
    return in_maps, cores


def _host_weights(cfg, fc_w, fc_b, dir_w, dir_b, neu_w, neu_b, rob_w, rob_b):
    import ml_dtypes

    D = cfg["D"]
    wcat = np.zeros((D + 1, 4 * D), np.float32)
    for t, (w, bb) in enumerate([(dir_w, dir_b), (neu_w, neu_b),
                                 (rob_w, rob_b), (fc_w, fc_b)]):
        wcat[:D, t * D:(t + 1) * D] = np.asarray(w, np.float32).T
        wcat[D, t * D:(t + 1) * D] = np.asarray(bb, np.float32)
    # h2 = x @ fc_w.T + 2*fc_b absorbs the deg*fc_b term from the col sum
    wcat[D, 3 * D:4 * D] *= 2.0
    wfcT = np.ascontiguousarray(np.asarray(fc_w, np.float32).T).astype(
        np.float16)  # [d, d'] so  S @ fc_w.T = (S.T as lhsT) x wfcT
    ident8 = np.eye(P, dtype=ml_dtypes.float8_e4m3)
    return wcat, wfcT, ident8


def _build_nc(cfg):
    import concourse.bass as bass
    import concourse.bacc as bacc
    import concourse.mybir as mybir
    import concourse.tile as tile
    from concourse.masks import make_identity

    D = cfg["D"]
    NBLK, NLOC_PAD = cfg["NBLK"], cfg["NLOC_PAD"]
    NTAB = cfg["NTAB"]
    K_total, colw, GROUP = cfg["K_total"], cfg["colw"], cfg["GROUP"]
    f32, f16, i32 = mybir.dt.float32, mybir.dt.float16, mybir.dt.int32
    bf16 = mybir.dt.bfloat16
    f8 = mybir.dt.float8e4
    coff = np.zeros(NBLK, np.int64)
    np.cumsum(np.asarray(colw[:-1]), out=coff[1:])

    nc = bacc.Bacc(dynamic_dma_scratch_size=cfg.get("DMASCRATCH",
                                                    16384))
    x8_d = nc.declare_dram_parameter("x8", [NTAB, D], f8, isOutput=False)
    xt_loc_d = nc.declare_dram_parameter("xt_loc", [D + 1, NLOC_PAD], f32,
                                         isOutput=False)
    eidx_d = nc.declare_dram_parameter("eidx", [P, K_total], i32,
                                       isOutput=False)
    degm_d = nc.declare_dram_parameter("degm", [P, NBLK], f32, isOutput=False)
    wcat_d = nc.declare_dram_parameter("wcat", [D + 1, 4 * D], f32,
                                       isOutput=False)
    wfcT_d = nc.declare_dram_parameter("wfcT", [D, D], f16, isOutput=False)
    ident8_d = nc.declare_dram_parameter("ident8", [P, P], f8, isOutput=False)
    y_d = nc.declare_dram_parameter("y", [P, NBLK * D], bf16,
                                    isOutput=True)

    # edge groups: <=GROUP blocks and bounded gather width per group
    groups, cur, csum = [], [], 0
    for b in range(NBLK):
        if cur and (csum + colw[b] > 17 * GROUP or len(cur) >= GROUP):
            groups.append(cur)
            cur, csum = [], 0
        cur.append(b)
        csum += colw[b]
    if cur:
        groups.append(cur)
    NG = len(groups)

    with tile.TileContext(nc) as tc:
        with (
            tc.tile_pool(name="const", bufs=1) as cp,
            tc.tile_pool(name="xtl", bufs=3) as xtlp,
            tc.tile_pool(name="msg", bufs=4) as mp,
            tc.tile_pool(name="ssb", bufs=2) as sp,
            tc.tile_pool(name="stb", bufs=2) as stp,
            tc.tile_pool(name="tmp", bufs=2) as tp,
            tc.tile_pool(name="osb", bufs=2) as op,
            tc.tile_pool(name="ps1", bufs=2, space="PSUM") as pp1,
            tc.tile_pool(name="ps2", bufs=2, space="PSUM") as pp2,
            tc.tile_pool(name="psT", bufs=2, space="PSUM") as ppT,
            tc.tile_pool(name="psA", bufs=2, space="PSUM") as ppA,
        ):
            # loop-invariant constants: loaded once, resident across LOOPR
            eidx_sb = cp.tile([P, K_total], i32)
            nc.sync.dma_start(out=eidx_sb[:], in_=eidx_d[:])
            wcat = cp.tile([D + 1, 4 * D], f32)
            nc.sync.dma_start(out=wcat[:], in_=wcat_d[:])
            wfcT = cp.tile([D, D], f16)
            nc.sync.dma_start(out=wfcT[:], in_=wfcT_d[:])
            ident8 = cp.tile([P, P], f8)
            nc.sync.dma_start(out=ident8[:], in_=ident8_d[:])
            degm_sb = cp.tile([P, NBLK], f32)
            nc.sync.dma_start(out=degm_sb[:], in_=degm_d[:])
            ident16 = cp.tile([P, P], f16)
            make_identity(nc, ident16[:])
            abgh = cp.tile([P, NBLK * 4 * D], f32)
            abgh3 = abgh[:].rearrange("p (t c) -> p t c", c=4 * D)

            def _bodyfn():

                # all gathers up-front on the Pool queue; msg pool bufs
                # pipeline them against PE consumption
                msgs = []
                for gi, blocks in enumerate(groups):
                    b0 = blocks[0]
                    goff = int(coff[b0])
                    Kg = int(sum(colw[b] for b in blocks))
                    msg = mp.tile([P, 17 * GROUP * D], f8, tag="msg")
                    nc.gpsimd.indirect_dma_start(
                        out=msg[:, :Kg * D], out_offset=None,
                        in_=x8_d[:],
                        in_offset=bass.IndirectOffsetOnAxis(
                            ap=eidx_sb[:, goff:goff + Kg], axis=0),
                    )
                    msgs.append(msg)

                # phase-1b work list: alpha/beta/gamma/h2 for pairs of blocks,
                # interleaved between gather groups to fill PE gaps
                XCH = 8
                xt_tiles = {}

                def emit_1b_pair(pi):
                    t0 = 2 * pi
                    c0 = (t0 // XCH) * XCH
                    if c0 not in xt_tiles:
                        nb_c = min(XCH, NBLK - c0)
                        xt = xtlp.tile([D + 1, XCH * P], f32, tag="xtl")
                        nc.sync.dma_start(
                            out=xt[:, :nb_c * P],
                            in_=xt_loc_d[:, P * c0:P * (c0 + nb_c)])
                        xt_tiles[c0] = xt
                    xt = xt_tiles[c0]
                    ps = pp1.tile([P, 2 * 4 * D], f32, tag="ps1b")
                    for j in range(2):
                        t = t0 + j
                        if t >= NBLK:
                            continue
                        nc.tensor.matmul(
                            out=ps[:, j * 4 * D:(j + 1) * 4 * D],
                            lhsT=xt[:, P * (t - c0):P * (t - c0 + 1)],
                            rhs=wcat[:], start=True, stop=True,
                            skip_group_check=True)
                    nlive = min(2, NBLK - t0)
                    ps3 = ps[:].rearrange("p (t c) -> p t c", c=4 * D)
                    out3 = abgh3[:, t0:t0 + nlive]
                    # relu on alpha|beta (EPS folded into den later); both
                    # halves on the Activation engine to keep DVE free
                    nc.scalar.activation(
                        out=out3[:, :, 0:2 * D], in_=ps3[:, :nlive, 0:2 * D],
                        func=mybir.ActivationFunctionType.Relu)
                    nc.scalar.copy(out=out3[:, :, 2 * D:4 * D],
                                   in_=ps3[:, :nlive, 2 * D:4 * D])

                NPAIR = (NBLK + 1) // 2
                pairs_per_g = -(-NPAIR // NG)
                next_pair = 0

                # steady-state pipeline over groups
                state = {}

                def emit_segsum(gi):
                    blocks = groups[gi]
                    nb = len(blocks)
                    msg = msgs[gi]
                    ps = pp2.tile([P, GROUP * 2 * D], f32, tag="psagg")
                    kk = 0
                    for bi, b in enumerate(blocks):
                        w = colw[b]
                        nj = (w + 1) // 2
                        for j in range(nj):
                            ncols = min(2, w - 2 * j)
                            nc.tensor.matmul(
                                out=ps[:, (2 * bi) * D:(2 * bi + ncols) * D],
                                lhsT=ident8[:],
                                rhs=msg[:, (kk + 2 * j) * D:
                                        (kk + 2 * j + ncols) * D],
                                start=(j == 0), stop=(j == nj - 1),
                                skip_group_check=True)
                        kk += w
                    # combine lanes -> S in f16 (copy + add; one PSUM input
                    # per DVE instruction)
                    ps4 = ps[:].rearrange("p (t l c) -> p t l c", l=2, c=D)
                    s_sb = sp.tile([P, GROUP * D], f16, tag="ssb")
                    s3 = s_sb[:].rearrange("p (t c) -> p t c", c=D)
                    nc.vector.tensor_copy(out=s3[:, :nb], in_=ps4[:, :nb, 0])
                    nc.vector.tensor_tensor(out=s3[:, :nb], in0=s3[:, :nb],
                                            in1=ps4[:, :nb, 1],
                                            op=mybir.AluOpType.add)
                    state[gi] = dict(s_sb=s_sb, nb=nb, blocks=blocks)

                def emit_finish(gi):
                    st = state.pop(gi)
                    s_sb, nb, blocks = st["s_sb"], st["nb"], st["blocks"]
                    b0 = blocks[0]
                    # transpose S per block:  psT[:, bi] = S_bi.T
                    psT = ppT.tile([2 * D, GROUP * P], f32, tag="psT")
                    for bi in range(nb):
                        nc.tensor.matmul(
                            out=psT[:D, bi * P:(bi + 1) * P],
                            lhsT=s_sb[:, bi * D:(bi + 1) * D],
                            rhs=ident16[:], start=True, stop=True,
                            skip_group_check=True)
                    st_sb = stp.tile([D, GROUP * P], f16, tag="stb")
                    nc.scalar.copy(out=st_sb[:, :nb * P], in_=psT[:D, :nb * P])
                    # apply fc_w:  A_bi = S_bi @ fc_w.T
                    psA = ppA.tile([P, GROUP * D], f32, tag="psA")
                    for bi in range(nb):
                        nc.tensor.matmul(
                            out=psA[:, bi * D:(bi + 1) * D],
                            lhsT=st_sb[:, bi * P:(bi + 1) * P],
                            rhs=wfcT[:], start=True, stop=True,
                            skip_group_check=True)
                    # epilogue
                    a3 = psA[:].rearrange("p (t c) -> p t c", c=D)
                    num = tp.tile([P, GROUP * D], f32, tag="num")
                    den = tp.tile([P, GROUP * D], f32, tag="den")
                    num3 = num[:].rearrange("p (t c) -> p t c", c=D)
                    den3 = den[:].rearrange("p (t c) -> p t c", c=D)
                    bsl = abgh3[:, b0:b0 + nb, D:2 * D]
                    gsl = abgh3[:, b0:b0 + nb, 2 * D:3 * D]
                    asl = abgh3[:, b0:b0 + nb, 0:D]
                    degb = degm_sb[:, b0:b0 + nb].rearrange(
                        "p (t u) -> p t u", u=1).to_broadcast([P, nb, D])
                    nn3 = num3[:, :nb]
                    dd3 = den3[:, :nb]
                    # den = alpha + beta*deg + EPS (group-wide ops)
                    nc.vector.tensor_tensor(out=dd3, in0=bsl, in1=degb,
                                            op=mybir.AluOpType.mult)
                    nc.vector.tensor_tensor(out=dd3, in0=dd3, in1=asl,
                                            op=mybir.AluOpType.add)
                    nc.vector.tensor_scalar(
                        out=den[:, :nb * D], in0=den[:, :nb * D],
                        scalar1=EPS, scalar2=None, op0=mybir.AluOpType.add)
                    nc.vector.reciprocal(out=dd3, in_=dd3)
                    # num = beta * (deg*h2 + S@fc_w.T) + gamma
                    # deg*h2 on the Activation engine (per-partition scale)
                    for bi, b in enumerate(blocks):
                        nc.scalar.mul(out=num[:, bi * D:(bi + 1) * D],
                                      in_=abgh3[:, b, 3 * D:4 * D],
                                      mul=degm_sb[:, b:b + 1])
                    nc.vector.tensor_tensor(out=nn3, in0=nn3, in1=a3[:, :nb],
                                            op=mybir.AluOpType.add)
                    nc.vector.tensor_tensor(out=nn3, in0=nn3, in1=bsl,
                                            op=mybir.AluOpType.mult)
                    nc.vector.tensor_tensor(out=nn3, in0=nn3, in1=gsl,
                                            op=mybir.AluOpType.add)
                    osb = op.tile([P, GROUP * D], bf16, tag="osb")
                    osb3 = osb[:].rearrange("p (t c) -> p t c", c=D)
                    nc.vector.tensor_tensor(out=osb3[:, :nb], in0=nn3,
                                            in1=dd3, op=mybir.AluOpType.mult)
                    nc.sync.dma_start(
                        out=y_d[:, b0 * D:(b0 + nb) * D], in_=osb[:, :nb * D])

                prev = None
                for gi in range(NG):
                    for _ in range(pairs_per_g):
                        if next_pair < NPAIR:
                            emit_1b_pair(next_pair)
                            next_pair += 1
                    # finish the previous group before the next segsum so its
                    # apply-matmul (psA) lands ahead of the new PE burst
                    if prev is not None:
                        emit_finish(prev)
                    emit_segsum(gi)
                    prev = gi
                while next_pair < NPAIR:
                    emit_1b_pair(next_pair)
                    next_pair += 1
                emit_finish(prev)

            LOOPR = cfg.get("LOOPR", 0)
            if LOOPR:
                with tc.For_i(0, LOOPR, 1) as _i:
                    _bodyfn()
            else:
                _bodyfn()
    nc.finalize()
    return nc


_BUILD_CACHE = {}
LAST_PROFILE = {}


def _get_runner(cfg):
    """Compile the bass program once; return an executor over 8 cores.

    Mirrors concourse.bass2jax.run_bass_via_pjrt's multi-core branch but
    caches the jitted callable so repeated executions don't re-trace."""
    key = (cfg["N"], cfg["NCORES"], tuple(cfg["colw"]), cfg["GROUP"],
           cfg.get("LOOPR", 0), cfg.get("DMASCRATCH", 16384))
    if key in _BUILD_CACHE:
        return _BUILD_CACHE[key]

    import jax
    import concourse.mybir as mybir
    from jax.experimental.shard_map import shard_map
    from jax.sharding import Mesh, PartitionSpec
    from concourse.bass2jax import (
        _bass_exec_p, install_neuronx_cc_hook, partition_id_tensor)

    nc = _build_nc(cfg)
    install_neuronx_cc_hook()
    n_cores = cfg["NCORES"]
    partition_name = (nc.partition_id_tensor.name
                      if nc.partition_id_tensor else None)
    in_names, out_names, out_avals, zero_outs = [], [], [], []
    for alloc in nc.m.functions[0].allocations:
        if not isinstance(alloc, mybir.MemoryLocationSet):
            continue
        name = alloc.memorylocations[0].name
        if alloc.kind == "ExternalInput":
            if name != partition_name:
                in_names.append(name)
        elif alloc.kind == "ExternalOutput":
            out_names.append(name)
            shape = tuple(alloc.tensor_shape)
            dtype = mybir.dt.np(alloc.dtype)
            out_avals.append(jax.core.ShapedArray(shape, dtype))
            zero_outs.append(np.zeros(shape, dtype))
    n_params = len(in_names)
    n_outs = len(out_avals)
    all_names = in_names + out_names
    if partition_name is not None:
        all_names.append(partition_name)

    def _body(*args):
        operands = list(args)
        if partition_name is not None:
            operands.append(partition_id_tensor())
        return tuple(_bass_exec_p.bind(
            *operands,
            out_avals=tuple(out_avals),
            in_names=tuple(all_names),
            out_names=tuple(out_names),
            lowering_input_output_aliases=(),
            sim_require_finite=True,
            sim_require_nnan=True,
            nc=nc,
        ))

    devices = jax.devices()[:n_cores]
    mesh = Mesh(np.asarray(devices), ("core",))
    in_specs = (PartitionSpec("core"),) * (n_params + n_outs)
    out_specs = (PartitionSpec("core"),) * n_outs
    donate = tuple(range(n_params, n_params + n_outs))
    sharded = jax.jit(
        shard_map(_body, mesh=mesh, in_specs=in_specs, out_specs=out_specs,
                  check_rep=False),
        donate_argnums=donate, keep_unused=True)

    import jax.numpy as jnp

    from jax.sharding import NamedSharding
    _zshard = tuple(NamedSharding(mesh, PartitionSpec("core"))
                    for _ in zero_outs)

    @functools.partial(jax.jit, out_shardings=_zshard)
    def _mkzeros():
        return tuple(jnp.zeros((n_cores * z.shape[0], *z.shape[1:]), z.dtype)
                     for z in zero_outs)

    def run(in_maps, reps=1, async_reps=0):
        import time as _time
        per_core = [[np.asarray(m[n]) for n in in_names] for m in in_maps]
        concat_in = [np.concatenate([per_core[c][i] for c in range(n_cores)],
                                    axis=0) for i in range(n_params)]
        concat_in = [jax.device_put(a) for a in concat_in]
        for a in concat_in:
            a.block_until_ready()
        times = []
        out_arrs = None
        for _ in range(max(1, reps)):
            concat_zeros = _mkzeros()
            for z in concat_zeros:
                z.block_until_ready()
            t0 = _time.perf_counter()
            out_arrs = sharded(*concat_in, *concat_zeros)
            for o in out_arrs:
                o.block_until_ready()
            times.append(_time.perf_counter() - t0)
        if async_reps:
            zsets = []
            for _ in range(async_reps):
                zs = _mkzeros()
                for z in zs:
                    z.block_until_ready()
                zsets.append(zs)
            t0 = _time.perf_counter()
            pend = [sharded(*concat_in, *zs) for zs in zsets]
            for oa in pend:
                for o in oa:
                    o.block_until_ready()
            times.append(("async_avg",
                          (_time.perf_counter() - t0) / async_reps))
        results = [
            {name: np.asarray(out_arrs[i]).reshape(n_cores,
                                                   *out_avals[i].shape)[c]
             for i, name in enumerate(out_names)}
            for c in range(n_cores)
        ]
        return results, times

    _BUILD_CACHE[key] = run
    return run


def _prepare(cfg, x, edge_index, degree, fc_w, fc_b, dir_w, dir_b,
             neu_w, neu_b, rob_w, rob_b):
    x = np.asarray(x)
    in_maps, cores = _host_prep(cfg, x, edge_index, degree)
    wcat, wfcT, ident8 = _host_weights(cfg, fc_w, fc_b, dir_w, dir_b,
                                       neu_w, neu_b, rob_w, rob_b)
    for im in in_maps:
        im["wcat"] = wcat
        im["wfcT"] = wfcT
        im["ident8"] = ident8
    return in_maps, cores


def _unshard(cfg, results, cores):
    N, D, NLOC, NBLK = cfg["N"], cfg["D"], cfg["NLOC"], cfg["NBLK"]
    out = np.empty((N, D), np.float32)
    for k in range(cfg["NCORES"]):
        y2 = np.asarray(results[k]["y"], np.float32).reshape(P, NBLK, D)
        y = np.ascontiguousarray(y2.transpose(1, 0, 2)).reshape(-1, D)[:NLOC]
        cc = cores[k]
        out[cc["base"] + cc["perm"]] = y
    return out


def kernel(x, edge_index, degree, fc_w, fc_b, dir_w, dir_b,
           neu_w, neu_b, rob_w, rob_b, _cfg=None, _reps=1, _async=0):
    cfg = _derive(dict(_cfg) if _cfg is not None else _cfg_full())
    in_maps, cores = _prepare(cfg, x, edge_index, degree, fc_w, fc_b,
                              dir_w, dir_b, neu_w, neu_b, rob_w, rob_b)
    run = _get_runner(cfg)
    results, times = run(in_maps, reps=_reps, async_reps=_async)
    LAST_PROFILE.clear()
    LAST_PROFILE["wall_times_s"] = times
    sync_times = [t for t in times if not isinstance(t, tuple)]
    LAST_PROFILE["exec_time_ns"] = int(min(sync_times) * 1e9)
    return _unshard(cfg, results, cores)
